# revision 1
# baseline (speedup 1.0000x reference)
"""GCN (2-layer, GCNConv-style with self-loops + symmetric normalization)
on 8 Trainium2 NeuronCores.

Strategy (graph/data parallel, per the sharding hint):
  - Nodes are permuted (degree-sorted, striped across cores) and sharded:
    core c owns padded rows [c*NPC_PAD, (c+1)*NPC_PAD).
  - Each core computes z1' = dinv * (x @ W1) for its nodes (PE matmuls,
    fp16), then an AllGather builds the full node-feature table in HBM.
  - Edges live on the core owning their destination. The halo exchange is
    the AllGather; per destination-block of 128 nodes the core gathers
    source rows with dma_gather (<=1024 rows per call — the SWDGE ring is
    1024 descriptors — round-robined over 4 SWDGE queues) and segment-sums
    them on the TensorEngine via identity-matmul PSUM accumulation. The
    GCN norm is folded in: dinv[src] into the table rows, dinv[dst] into
    the PSUM-evacuation scale, and the bias enters as a rank-1
    outer-product matmul (sqrt(deg)[dst] x b) in the same PSUM group.
  - dma_gather indices are int16 (table rows < 32768), so the 50176-row
    table is addressed through NV=3 overlapping 32768-row windows; the
    host waterfills each destination's edges across the windows
    (earliest-deadline-first) to minimize per-block slot padding.
  - Layer 2 repeats the pattern with z2' = dinv * (h1 @ W2) (table rows
    padded to 256B, but only the first 96B fetched per descriptor),
    reusing the same gather indices, and ends with a fused log_softmax.
"""
import sys

sys.path.insert(0, "/opt/trn_rl_repo")

import numpy as np

import concourse.bass as bass
import concourse.bacc as bacc
import concourse.tile as tile
import concourse.mybir as mybir
from concourse import bass_utils

P = 128
NC = 8
F16 = mybir.dt.float16
F32 = mybir.dt.float32
I16 = mybir.dt.int16
MAX_CALL_SLOTS = 8          # 1024 rows = HW SWDGE descriptor-ring capacity


class Cfg:
    def __init__(self, N, F_IN, F_HID, F_OUT, view_rows=32768, slot_cap=112,
                 phase=4, single_core=False, nv=3):
        self.phase = phase
        self.single_core = single_core
        self.N = N
        self.F_IN = F_IN
        self.F_HID = F_HID
        self.F_OUT = F_OUT
        self.FO_PAD = F_OUT + 1          # one pad col for log_softmax masking
        self.KC = (F_IN + P - 1) // P
        self.K_PAD = self.KC * P
        self.NB = (N + NC * P - 1) // (NC * P)
        self.NPC_PAD = self.NB * P
        self.TOT = NC * self.NPC_PAD
        self.VIEW_ROWS = view_rows
        self.NV = 1 if self.TOT <= view_rows else nv
        if self.NV == 1:
            self.BASES = [0]
        else:
            self.BASES = [round(v * (self.TOT - view_rows) / (self.NV - 1))
                          for v in range(self.NV)]
        assert self.BASES[-1] + view_rows >= self.TOT
        self.SLOT_CAP = slot_cap


def _preprocess(x, W1, b1, W2, b2, edge_index, cfg):
    N, NB, NPC_PAD, TOT = cfg.N, cfg.NB, cfg.NPC_PAD, cfg.TOT
    NV, V, BASES = cfg.NV, cfg.VIEW_ROWS, cfg.BASES
    src = np.asarray(edge_index[0], dtype=np.int64)
    dst = np.asarray(edge_index[1], dtype=np.int64)

    indeg = np.bincount(dst, minlength=N)
    deg = (indeg + 1).astype(np.float64)
    dinv = (1.0 / np.sqrt(deg)).astype(np.float32)
    rdinv = np.sqrt(deg).astype(np.float32)

    # ---- node permutation: degree-sorted desc, striped over cores.
    # One "hole" (guaranteed-zero row) is reserved inside every view window
    # to serve as gather padding.
    hole_cores = sorted({min((BASES[v] + V // 2) // NPC_PAD, NC - 1)
                         for v in range(NV)})
    hole_ranks = sorted(((NB - 1) * NC + c) * P + 127 for c in hole_cores)
    assert TOT - len(hole_ranks) >= N
    order = np.argsort(-deg, kind="stable")
    ii = np.arange(N)
    for h in hole_ranks:
        ii = np.where(ii >= h, ii + 1, ii)
    gi, pi = ii // P, ii % P
    assert gi.max() // NC < NB
    pos_of = np.empty(N, dtype=np.int64)
    pos_of[order] = (gi % NC) * NPC_PAD + (gi // NC) * P + pi
    hole_rows = [c * NPC_PAD + NPC_PAD - 1 for c in hole_cores]
    pad_row = []
    for v in range(NV):
        cands = [r for r in hole_rows if BASES[v] <= r < BASES[v] + V]
        assert cands, (v, BASES, hole_rows)
        pad_row.append(cands[0])

    # ---- edge stream incl. self-loops, sorted by destination position
    ps = np.concatenate([pos_of[src], pos_of])
    pd = np.concatenate([pos_of[dst], pos_of])
    eo = np.argsort(pd, kind="stable")
    ps, pd = ps[eo], pd[eo]
    E2 = len(ps)

    # ---- view windows: waterfill each dst's edges across views
    ends = np.array([b + V for b in BASES])
    vhi = np.searchsorted(np.array(BASES), ps, side="right") - 1
    vlo = np.searchsorted(ends, ps, side="right")
    d_tot = np.bincount(pd, minlength=TOT)

    assigned = np.full(E2, -1, dtype=np.int8)
    remaining = d_tot.astype(np.int64).copy()
    n_view = np.zeros((NV, TOT), dtype=np.int64)
    for v in range(NV):
        un = assigned < 0
        must = un & (vhi == v)
        may = un & (vlo <= v) & (vhi > v)
        cnt_must = np.bincount(pd[must], minlength=TOT)
        cnt_may = np.bincount(pd[may], minlength=TOT)
        T = -(-remaining // (NV - v))       # ceil
        take_may = np.clip(T - cnt_must, 0, cnt_may)
        # rank may-edges within dst, earliest-deadline (vhi asc) first
        mi = np.flatnonzero(may)
        mo = mi[np.lexsort((vhi[mi], pd[mi]))]
        pdm = pd[mo]
        st = np.flatnonzero(np.r_[True, pdm[1:] != pdm[:-1]])
        mrank = np.arange(len(mo)) - np.repeat(st, np.diff(np.r_[st, len(mo)]))
        sel = mo[mrank < take_may[pdm]]
        assigned[must] = v
        assigned[sel] = v
        n_view[v] = cnt_must + np.minimum(take_may, cnt_may)
        remaining -= n_view[v]
    assert (assigned >= 0).all() and (remaining == 0).all()

    # ---- per-block scheduled slots (shared across cores)
    S = np.zeros((NV, NB), dtype=np.int64)
    for v in range(NV):
        S[v] = n_view[v].reshape(TOT // P, P).max(axis=1).reshape(NC, NB).max(0)
    oV = np.zeros((NV, NB + 1), dtype=np.int64)
    for v in range(NV):
        oV[v, 1:] = np.cumsum(S[v])
    TV = oV[:, -1]

    # ---- index arrays
    arrs = []
    for v in range(NV):
        arr = np.full((NC, int(TV[v]) * P), pad_row[v] - BASES[v],
                      dtype=np.int32)
        ei = np.flatnonzero(assigned == v)
        pde = pd[ei]
        st = np.flatnonzero(np.r_[True, pde[1:] != pde[:-1]])
        jj = np.arange(len(ei)) - np.repeat(st, np.diff(np.r_[st, len(ei)]))
        core = pde // NPC_PAD
        blk = (pde % NPC_PAD) // P
        pp = pde % P
        flat = (oV[v, blk] + jj) * P + pp
        arr[core, flat] = ps[ei] - BASES[v]
        assert arr.min() >= 0 and arr.max() < V
        arrs.append(arr)

    # ---- chunk packing
    chunks = []
    cur, tot_s = [], 0
    for b in range(NB):
        s = int(S[:, b].sum())
        if cur and tot_s + s > cfg.SLOT_CAP:
            chunks.append(cur)
            cur, tot_s = [], 0
        cur.append(b)
        tot_s += s
    if cur:
        chunks.append(cur)

    def wrap16(arr):   # stream position q -> [q%16, q//16], replicated 8x
        w = arr.reshape(NC, -1, 16).transpose(0, 2, 1).astype(np.int16)
        return np.tile(w, (1, 8, 1))

    idxs = [wrap16(a) for a in arrs]

    # ---- per-position node data
    xp = np.zeros((TOT, cfg.K_PAD), dtype=np.float16)
    xp[pos_of, : cfg.F_IN] = np.asarray(x, np.float32).astype(np.float16)
    dinv_pos = np.zeros(TOT, dtype=np.float32)
    dinv_pos[pos_of] = dinv
    rdinv_pos = np.zeros(TOT, dtype=np.float32)
    rdinv_pos[pos_of] = rdinv

    W1p = np.zeros((cfg.K_PAD, cfg.F_HID), dtype=np.float16)
    W1p[: cfg.F_IN] = np.asarray(W1, np.float32).astype(np.float16)
    W2p = np.zeros((cfg.F_HID, P), dtype=np.float16)
    W2p[:, : cfg.F_OUT] = np.asarray(W2, np.float32).astype(np.float16)
    b1row = np.asarray(b1, np.float32).astype(np.float16).reshape(1, cfg.F_HID)
    b2row = np.zeros((1, P), dtype=np.float16)
    b2row[0, : cfg.F_OUT] = np.asarray(b2, np.float32).astype(np.float16)
    b2row[0, cfg.F_OUT: cfg.FO_PAD] = -60000.0
    ident = np.eye(P, dtype=np.float16)

    in_maps = []
    for c in range(NC):
        xc = xp[c * NPC_PAD:(c + 1) * NPC_PAD]
        m = {
            "xT": np.ascontiguousarray(xc.T).reshape(cfg.KC, P, NPC_PAD),
            "W1p": W1p.reshape(cfg.KC, P, cfg.F_HID),
            "W2p": W2p,
            "b1row": b1row,
            "b2row": b2row,
            "dinvc": np.ascontiguousarray(
                dinv_pos[c * NPC_PAD:(c + 1) * NPC_PAD].reshape(NB, P).T),
            "rdinvT": rdinv_pos[c * NPC_PAD:(c + 1) * NPC_PAD]
                      .reshape(1, NPC_PAD).astype(np.float16),
            "ident": ident,
        }
        for v in range(NV):
            m[f"idx{v}"] = idxs[v][c]
        in_maps.append(m)

    sched = {
        "S": S.tolist(), "oV": oV.tolist(), "TV": [int(t) for t in TV],
        "chunks": chunks,
    }
    return in_maps, sched, pos_of


def _dma_gather_narrow(gps, out_ap, in_ap, idxs_ap, num_idxs, num_idxs_reg,
                       elem_size, elem_step, queue_num=0):
    """dma_gather without the %256B elem-size restriction (non-transpose,
    DRAM source). The 256B-granularity constraint is on the row stride
    (stride_bytes_256 field), not the payload size. HW-verified (smoke9)."""
    from concourse import ap_utils
    gps._assert_queue_num(queue_num)
    assert idxs_ap.dtype == mybir.dt.int16
    assert in_ap.space == bass.MemorySpace.DRAM
    assert in_ap.dtype == out_ap.dtype
    assert ap_utils.ap_is_contiguous(out_ap.ap[1:])
    assert ap_utils.ap_is_contiguous(idxs_ap.ap[1:])
    assert in_ap.ap[0][0] == elem_step
    stride_bytes = elem_step * mybir.dt.size(in_ap.dtype)
    assert stride_bytes % 256 == 0 and stride_bytes // 256 < 256
    assert in_ap.ap[-1][1] == elem_size
    assert out_ap.ap[-1][1] == elem_size
    assert num_idxs % P == 0
    assert out_ap.ap[0][1] * out_ap.ap[1][1] == num_idxs
    _in_ap = gps.lower_ap_dma(in_ap, for_custom_bir_dma=True)
    _idxs_ap = gps.lower_ap(idxs_ap)
    _out_ap = gps.lower_ap(out_ap)
    return gps.add_instruction(
        mybir.InstDMAGatherAnt(
            name=gps.bass.get_next_instruction_name(),
            ins=[*_in_ap, _idxs_ap, gps.lower_val_access(gps.to_reg(num_idxs_reg))],
            outs=[_out_ap],
            transpose=False, num_idxs=num_idxs, elem_size=elem_size,
            stride_bytes_256=stride_bytes // 256, gen_mode=0,
            single_packet=True, queue_num=queue_num,
            sbuf_tokens_per_rank=0, sbuf_free_dim_per_rank=0,
            sbuf_free_dim_pad_per_rank=0, sbuf_byte_offset=0,
        ))


def _build_program(cfg, sch):
    NB, NPC_PAD, TOT = cfg.NB, cfg.NPC_PAD, cfg.TOT
    FH, KC, NV = cfg.F_HID, cfg.KC, cfg.NV
    S = sch["S"]
    oV = sch["oV"]

    nc = bacc.Bacc("TRN2", target_bir_lowering=False, debug=False,
                   num_devices=1 if cfg.single_core else NC,
                   num_swdge_queues=4)
    xT_in = nc.dram_tensor("xT", [KC, P, NPC_PAD], F16, kind="ExternalInput")
    W1_in = nc.dram_tensor("W1p", [KC, P, FH], F16, kind="ExternalInput")
    W2_in = nc.dram_tensor("W2p", [FH, P], F16, kind="ExternalInput")
    b1_in = nc.dram_tensor("b1row", [1, FH], F16, kind="ExternalInput")
    b2_in = nc.dram_tensor("b2row", [1, P], F16, kind="ExternalInput")
    dinv_in = nc.dram_tensor("dinvc", [P, NB], F32, kind="ExternalInput")
    rdinv_in = nc.dram_tensor("rdinvT", [1, NPC_PAD], F16, kind="ExternalInput")
    idx_in = [nc.dram_tensor(f"idx{v}", [P, sch["TV"][v] * P // 16], I16,
                             kind="ExternalInput") for v in range(NV)]
    id_in = nc.dram_tensor("ident", [P, P], F16, kind="ExternalInput")
    out_dram = nc.dram_tensor("out", [NPC_PAD, cfg.FO_PAD], F32,
                              kind="ExternalOutput")

    rg = [list(range(NC))]

    with tile.TileContext(nc) as tc:
        with tc.tile_pool(name="sb", bufs=1) as sb, \
             tc.tile_pool(name="ps", bufs=1, space="PSUM") as ps, \
             tc.tile_pool(name="dram", bufs=1, space="DRAM") as dram:

            # --- constant loads -------------------------------------------
            xT_t, W1_t, xT_free = [], [], []
            for k in range(KC):
                xk, xfree = tc.tile([P, NPC_PAD], F16, name=f"xT_t{k}")
                nc.sync.dma_start(xk[:], xT_in.ap()[k])
                xT_t.append(xk)
                xT_free.append(xfree)
                wk = sb.tile([P, FH], F16, name=f"W1_t{k}")
                nc.sync.dma_start(wk[:], W1_in.ap()[k])
                W1_t.append(wk)
            W2_t = sb.tile([FH, P], F16, name="W2_t")
            nc.sync.dma_start(W2_t[:], W2_in.ap())
            b1_t = sb.tile([1, FH], F16, name="b1_t")
            nc.sync.dma_start(b1_t[:], b1_in.ap())
            b2_t = sb.tile([1, P], F16, name="b2_t")
            nc.sync.dma_start(b2_t[:], b2_in.ap())
            dinv_t = sb.tile([P, NB], F32, name="dinv_t")
            nc.sync.dma_start(dinv_t[:], dinv_in.ap())
            rdinv_t = sb.tile([1, NPC_PAD], F16, name="rdinv_t")
            nc.sync.dma_start(rdinv_t[:], rdinv_in.ap())
            idx_t = []
            for v in range(NV):
                it = sb.tile([P, sch["TV"][v] * P // 16], I16, name=f"idx_t{v}")
                nc.sync.dma_start(it[:], idx_in[v].ap())
                idx_t.append(it)
            id_t = sb.tile([P, P], F16, name="id_t")
            nc.sync.dma_start(id_t[:], id_in.ap())

            bounce1 = dram.tile([NPC_PAD, FH], F16, name="bounce1")
            table1 = dram.tile([TOT, FH], F16, name="table1")
            bounce2 = dram.tile([NPC_PAD, P], F16, name="bounce2")
            table2 = dram.tile([TOT, P], F16, name="table2")

            # --- layer-1 dense transform: z1' = dinv * (x @ W1) ----------
            z1_all = sb.tile([P, NB, FH], F16, name="z1_all")
            for b in range(NB):
                psz = ps.tile([P, FH], F32, tag="ps", bufs=4, name=f"psz{b}")
                for k in range(KC):
                    nc.tensor.matmul(out=psz[:],
                                     lhsT=xT_t[k][:, b * P:(b + 1) * P],
                                     rhs=W1_t[k][:],
                                     start=(k == 0), stop=(k == KC - 1))
                nc.scalar.activation(z1_all[:, b, :], psz[:],
                                     mybir.ActivationFunctionType.Copy,
                                     bias=0.0, scale=dinv_t[:, b:b + 1])
            nc.sync.dma_start(
                bounce1[:].rearrange("(nb p) f -> p nb f", p=P), z1_all[:])
            for f in reversed(xT_free):
                f()

            if cfg.phase >= 1:
                if cfg.single_core:
                    nc.sync.dma_start(table1[0:NPC_PAD, :], bounce1[:])
                else:
                    nc.gpsimd.collective_compute(
                        "AllGather", mybir.AluOpType.bypass, replica_groups=rg,
                        ins=[bounce1.opt()], outs=[table1.opt()])

            if cfg.phase <= 1:
                dbg = sb.tile([P, cfg.FO_PAD], F32, name="dbgout")
                nc.gpsimd.memset(dbg[:], 0.0)
                for b in range(NB):
                    nc.sync.dma_start(out_dram.ap()[b * P:(b + 1) * P, :], dbg[:])

            # --- generic gather + segment-reduce phase -------------------
            qctr = [0]

            def agg_phase(table, fw_row, fw_fetch, brow_t, fo, consume):
                """table rows are [*, fw_row] f16 (256B-multiple stride);
                each descriptor fetches the first fw_fetch cols; reduce fo
                cols per block into PSUM; consume(b, psum) finishes it."""
                views = [table[cfg.BASES[v]:cfg.BASES[v] + cfg.VIEW_ROWS,
                               0:fw_fetch] if NV > 1 else table[:, 0:fw_fetch]
                         for v in range(NV)]

                def gather_stream(g_tile, g_off, view, it, o0, n_slots):
                    sslot = 0
                    while sslot < n_slots:
                        m = min(MAX_CALL_SLOTS, n_slots - sslot)
                        _dma_gather_narrow(
                            nc.gpsimd,
                            out_ap=g_tile[:, g_off + sslot:g_off + sslot + m, :],
                            in_ap=view,
                            idxs_ap=it[:, (o0 + sslot) * 8:(o0 + sslot + m) * 8],
                            num_idxs=m * P, num_idxs_reg=m * P,
                            elem_size=fw_fetch, elem_step=fw_row,
                            queue_num=qctr[0] % 4)
                        qctr[0] += 1
                        sslot += m

                for ci, blocks in enumerate(sch["chunks"]):
                    b0, b1_ = blocks[0], blocks[-1]
                    nS = [oV[v][b1_ + 1] - oV[v][b0] for v in range(NV)]
                    g = sb.tile([P, sum(nS), fw_fetch], F16, tag="g", bufs=2,
                                name=f"g{fw_fetch}_{ci}")
                    go = np.r_[0, np.cumsum(nS)]
                    for v in range(NV):
                        if nS[v]:
                            gather_stream(g, int(go[v]), views[v], idx_t[v],
                                          oV[v][b0], nS[v])
                    for b in blocks:
                        pag = ps.tile([P, fo], F32, tag="ps", bufs=4,
                                      name=f"pag{fw_fetch}_{b}")
                        first = True
                        for v in range(NV):
                            for j in range(S[v][b]):
                                nc.tensor.matmul(
                                    out=pag[:], lhsT=id_t[:],
                                    rhs=g[:, int(go[v]) + oV[v][b] - oV[v][b0]
                                          + j, 0:fo],
                                    start=first, stop=False)
                                first = False
                        nc.tensor.matmul(
                            out=pag[:], lhsT=rdinv_t[:, b * P:(b + 1) * P],
                            rhs=brow_t[:, 0:fo], start=first, stop=True)
                        consume(b, pag)

            # --- layer-1 aggregation -> h1 -------------------------------
            if cfg.phase >= 2:
                h1_t = sb.tile([P, NB, FH], F16, name="h1_t")

                def l1_consume(b, pag):
                    nc.scalar.activation(h1_t[:, b, :], pag[:],
                                         mybir.ActivationFunctionType.Relu,
                                         bias=0.0, scale=dinv_t[:, b:b + 1])

                agg_phase(table1, FH, FH, b1_t, FH, l1_consume)

            if cfg.phase == 2:
                for b in range(NB):
                    o_t = sb.tile([P, cfg.FO_PAD], F32, tag="o", bufs=3,
                                  name=f"dbg{b}")
                    nc.vector.tensor_copy(o_t[:], h1_t[:, b, 0:cfg.FO_PAD])
                    nc.sync.dma_start(out_dram.ap()[b * P:(b + 1) * P, :], o_t[:])

            # --- layer-2 dense transform: z2' = dinv * (h1 @ W2) ---------
            if cfg.phase >= 3:
                z2_all = sb.tile([P, NB, P], F16, name="z2_all")
            for b in range(NB if cfg.phase >= 3 else 0):
                pst = ps.tile([P, P], F16, tag="pst", bufs=2, name=f"pst{b}")
                nc.tensor.transpose(out=pst[:], in_=h1_t[:, b, :],
                                    identity=id_t[:])
                h1T = sb.tile([P, P], F16, tag="h1T", bufs=3, name=f"h1T{b}")
                nc.scalar.activation(h1T[:], pst[:],
                                     mybir.ActivationFunctionType.Copy)
                psz2 = ps.tile([P, P], F32, tag="ps", bufs=4, name=f"psz2{b}")
                nc.tensor.matmul(out=psz2[:], lhsT=h1T[:], rhs=W2_t[:],
                                 start=True, stop=True)
                nc.scalar.activation(z2_all[:, b, :], psz2[:],
                                     mybir.ActivationFunctionType.Copy,
                                     bias=0.0, scale=dinv_t[:, b:b + 1])

            if cfg.phase >= 3:
                nc.sync.dma_start(
                    bounce2[:].rearrange("(nb p) f -> p nb f", p=P), z2_all[:])
                if cfg.single_core:
                    nc.sync.dma_start(table2[0:NPC_PAD, :], bounce2[:])
                else:
                    nc.gpsimd.collective_compute(
                        "AllGather", mybir.AluOpType.bypass, replica_groups=rg,
                        ins=[bounce2.opt()], outs=[table2.opt()])

            if cfg.phase == 3:
                dbg3 = sb.tile([P, cfg.FO_PAD], F32, name="dbgout3")
                nc.gpsimd.memset(dbg3[:], 0.0)
                for b in range(NB):
                    nc.sync.dma_start(out_dram.ap()[b * P:(b + 1) * P, :],
                                      dbg3[:])

            # --- layer-2 aggregation + log_softmax -----------------------
            if cfg.phase >= 4:
                out_all = sb.tile([P, NB, cfg.FO_PAD], F32, name="out_all")

            def l2_consume(b, pag):
                fo = cfg.FO_PAD
                m0 = sb.tile([P, 1], F32, tag="m0", bufs=3, name=f"m0_{b}")
                nc.vector.tensor_reduce(m0[:], pag[:], mybir.AxisListType.X,
                                        mybir.AluOpType.max)
                mneg = sb.tile([P, 1], F32, tag="mneg", bufs=3, name=f"mn{b}")
                nc.vector.tensor_scalar(mneg[:], m0[:], dinv_t[:, b:b + 1],
                                        -1.0, mybir.AluOpType.mult,
                                        mybir.AluOpType.mult)
                e_t = sb.tile([P, fo], F32, tag="e", bufs=3, name=f"e{b}")
                s_t = sb.tile([P, 1], F32, tag="s", bufs=3, name=f"s{b}")
                nc.scalar.activation(e_t[:], pag[:],
                                     mybir.ActivationFunctionType.Exp,
                                     bias=mneg[:], scale=dinv_t[:, b:b + 1],
                                     accum_out=s_t[:])
                lse = sb.tile([P, 1], F32, tag="lse", bufs=3, name=f"ls{b}")
                nc.scalar.activation(lse[:], s_t[:],
                                     mybir.ActivationFunctionType.Ln)
                c_t = sb.tile([P, 1], F32, tag="c", bufs=3, name=f"c{b}")
                nc.vector.tensor_tensor(out=c_t[:], in0=lse[:], in1=mneg[:],
                                        op=mybir.AluOpType.subtract)
                nc.vector.tensor_scalar(out_all[:, b, :], pag[:],
                                        dinv_t[:, b:b + 1],
                                        c_t[:], mybir.AluOpType.mult,
                                        mybir.AluOpType.subtract)

            if cfg.phase >= 4:
                agg_phase(table2, P, cfg.FO_PAD, b2_t, cfg.FO_PAD, l2_consume)
                nc.sync.dma_start(
                    out_dram.ap().rearrange("(nb p) f -> p nb f", p=P),
                    out_all[:])

    nc.compile()
    return nc


LAST_RESULTS = None


def kernel(x, W1, b1, W2, b2, edge_index):
    global LAST_RESULTS
    import os
    import time
    cfg = Cfg(N=50000, F_IN=500, F_HID=128, F_OUT=47,
              phase=int(os.environ.get("GCN_PHASE", "4")))
    in_maps, sched, pos_of = _preprocess(x, W1, b1, W2, b2, edge_index, cfg)
    nc = _build_program(cfg, sched)
    res = None
    for attempt in range(3):
        try:
            res = bass_utils.run_bass_kernel_spmd(
                nc, in_maps, core_ids=list(range(NC)))
            break
        except Exception:
            if attempt == 2:
                raise
            time.sleep(5)
    LAST_RESULTS = res
    alls = np.concatenate([np.asarray(res.results[c]["out"])
                           for c in range(NC)], axis=0)
    return alls[pos_of, : cfg.F_OUT].astype(np.float32)



# revision 23
# speedup vs baseline: 3.9058x; 3.9058x over previous
"""GCN (2-layer, GCNConv-style with self-loops + symmetric normalization)
on 8 Trainium2 NeuronCores.

Strategy (graph/data parallel, per the sharding hint):
  - Nodes are permuted (degree-sorted, striped across cores) and sharded:
    core c owns padded rows [c*NPC_PAD, (c+1)*NPC_PAD).
  - Each core computes z1' = dinv * (x @ W1) for its nodes (PE matmuls,
    fp16), then an AllGather builds the full node-feature table in HBM.
  - Edges live on the core owning their destination. The halo exchange is
    the AllGather; per destination-block of 128 nodes the core gathers
    source rows with dma_gather (<=1024 rows per call — the SWDGE ring is
    1024 descriptors — round-robined over 4 SWDGE queues) and segment-sums
    them on the TensorEngine via identity-matmul PSUM accumulation. The
    GCN norm is folded in: dinv[src] into the table rows, dinv[dst] into
    the PSUM-evacuation scale, and the bias enters as a rank-1
    outer-product matmul (sqrt(deg)[dst] x b) in the same PSUM group.
  - dma_gather indices are int16 (table rows < 32768), so the 50176-row
    table is addressed through NV=3 overlapping 32768-row windows; the
    host waterfills each destination's edges across the windows
    (earliest-deadline-first) to minimize per-block slot padding.
  - Layer 2 repeats the pattern with z2' = dinv * (h1 @ W2) (table rows
    padded to 256B, but only the first 96B fetched per descriptor),
    reusing the same gather indices, and ends with a fused log_softmax.
"""
import sys

sys.path.insert(0, "/opt/trn_rl_repo")

import numpy as np

import concourse.bass as bass
import concourse.bacc as bacc
import concourse.tile as tile
import concourse.mybir as mybir
from concourse import bass_utils

P = 128
NC = 8
F16 = mybir.dt.float16
F32 = mybir.dt.float32
I16 = mybir.dt.int16
MAX_CALL_SLOTS = 8          # 1024 rows = HW SWDGE descriptor-ring capacity


class Cfg:
    def __init__(self, N, F_IN, F_HID, F_OUT, view_rows=32768, slot_cap=112,
                 phase=4, single_core=False, nv=3, repeat=1, no_cc=False,
                 shared_tables=False, sv=True, no_self=True):
        self.phase = phase
        self.single_core = single_core
        self.repeat = repeat
        self.no_cc = no_cc
        self.shared_tables = shared_tables
        self.N = N
        self.F_IN = F_IN
        self.F_HID = F_HID
        self.F_OUT = F_OUT
        self.FO_PAD = F_OUT + 1          # one pad col for log_softmax masking
        self.KC = (F_IN + P - 1) // P
        self.K_PAD = self.KC * P
        self.NB = (N + NC * P - 1) // (NC * P)
        self.NPC_PAD = self.NB * P
        self.TOT = NC * self.NPC_PAD
        self.VIEW_ROWS = view_rows
        self.NV = 1 if self.TOT <= view_rows else nv
        if self.NV == 1:
            self.BASES = [0]
        else:
            self.BASES = [round(v * (self.TOT - view_rows) / (self.NV - 1))
                          for v in range(self.NV)]
        # signed single view: the gather ucode sign-extends int16 indices
        # (HW-verified), so a view based at row 32768 reaches rows
        # 0..65535 with idx in [-32768, 32767].
        self.sv = sv and self.NV > 1 and self.TOT <= 32768 + view_rows
        if self.sv:
            self.NV = 1
            # base so that max idx = TOT-1-base <= 32767 and most idx >= 0
            self.BASES = [self.TOT - view_rows]
        else:
            assert self.BASES[-1] + view_rows >= self.TOT
        self.no_self = no_self
        self.SLOT_CAP = slot_cap


def _preprocess(x, W1, b1, W2, b2, edge_index, cfg):
    N, NB, NPC_PAD, TOT = cfg.N, cfg.NB, cfg.NPC_PAD, cfg.TOT
    NV, V, BASES = cfg.NV, cfg.VIEW_ROWS, cfg.BASES
    src = np.asarray(edge_index[0], dtype=np.int64)
    dst = np.asarray(edge_index[1], dtype=np.int64)

    indeg = np.bincount(dst, minlength=N)
    deg = (indeg + 1).astype(np.float64)
    dinv = (1.0 / np.sqrt(deg)).astype(np.float32)
    rdinv = np.sqrt(deg).astype(np.float32)

    # ---- node permutation: degree-sorted desc, striped over cores.
    # One "hole" (guaranteed-zero row) is reserved inside every view window
    # to serve as gather padding.
    hole_cores = sorted({min((BASES[v] + V // 2) // NPC_PAD, NC - 1)
                         for v in range(NV)})
    hole_ranks = sorted(((NB - 1) * NC + c) * P + 127 for c in hole_cores)
    assert TOT - len(hole_ranks) >= N
    order = np.argsort(-deg, kind="stable")
    ii = np.arange(N)
    for h in hole_ranks:
        ii = np.where(ii >= h, ii + 1, ii)
    gi, pi = ii // P, ii % P
    assert gi.max() // NC < NB
    pos_of = np.empty(N, dtype=np.int64)
    pos_of[order] = (gi % NC) * NPC_PAD + (gi // NC) * P + pi
    hole_rows = [c * NPC_PAD + NPC_PAD - 1 for c in hole_cores]
    pad_row = []
    for v in range(NV):
        lo = BASES[v] - V if cfg.sv else BASES[v]
        cands = [r for r in hole_rows if lo <= r < BASES[v] + V]
        assert cands, (v, BASES, hole_rows)
        pad_row.append(cands[0])

    # ---- edge stream (self-loops handled in-PSUM when cfg.no_self),
    # sorted by destination position
    if cfg.no_self:
        ps, pd = pos_of[src], pos_of[dst]
    else:
        ps = np.concatenate([pos_of[src], pos_of])
        pd = np.concatenate([pos_of[dst], pos_of])
    eo = np.argsort(pd, kind="stable")
    ps, pd = ps[eo], pd[eo]
    E2 = len(ps)

    # ---- view windows: waterfill each dst's edges across views
    d_tot = np.bincount(pd, minlength=TOT)
    if NV == 1:
        assigned = np.zeros(E2, dtype=np.int8)
        n_view = d_tot.astype(np.int64).reshape(1, TOT)
    else:
        ends = np.array([b + V for b in BASES])
        vhi = np.searchsorted(np.array(BASES), ps, side="right") - 1
        vlo = np.searchsorted(ends, ps, side="right")

        assigned = np.full(E2, -1, dtype=np.int8)
        remaining = d_tot.astype(np.int64).copy()
        n_view = np.zeros((NV, TOT), dtype=np.int64)
        for v in range(NV):
            un = assigned < 0
            must = un & (vhi == v)
            may = un & (vlo <= v) & (vhi > v)
            cnt_must = np.bincount(pd[must], minlength=TOT)
            cnt_may = np.bincount(pd[may], minlength=TOT)
            T = -(-remaining // (NV - v))       # ceil
            take_may = np.clip(T - cnt_must, 0, cnt_may)
            # rank may-edges within dst, earliest-deadline (vhi asc) first
            mi = np.flatnonzero(may)
            mo = mi[np.lexsort((vhi[mi], pd[mi]))]
            pdm = pd[mo]
            st = np.flatnonzero(np.r_[True, pdm[1:] != pdm[:-1]])
            mrank = np.arange(len(mo)) - np.repeat(
                st, np.diff(np.r_[st, len(mo)]))
            sel = mo[mrank < take_may[pdm]]
            assigned[must] = v
            assigned[sel] = v
            n_view[v] = cnt_must + np.minimum(take_may, cnt_may)
            remaining -= n_view[v]
        assert (assigned >= 0).all() and (remaining == 0).all()

    # ---- per-block scheduled slots (shared across cores)
    S = np.zeros((NV, NB), dtype=np.int64)
    for v in range(NV):
        S[v] = n_view[v].reshape(TOT // P, P).max(axis=1).reshape(NC, NB).max(0)
    oV = np.zeros((NV, NB + 1), dtype=np.int64)
    for v in range(NV):
        oV[v, 1:] = np.cumsum(S[v])
    TV = oV[:, -1]

    # ---- index arrays
    arrs = []
    for v in range(NV):
        arr = np.full((NC, int(TV[v]) * P), pad_row[v] - BASES[v],
                      dtype=np.int32)
        ei = np.flatnonzero(assigned == v)
        pde = pd[ei]
        st = np.flatnonzero(np.r_[True, pde[1:] != pde[:-1]])
        jj = np.arange(len(ei)) - np.repeat(st, np.diff(np.r_[st, len(ei)]))
        core = pde // NPC_PAD
        blk = (pde % NPC_PAD) // P
        pp = pde % P
        flat = (oV[v, blk] + jj) * P + pp
        arr[core, flat] = ps[ei] - BASES[v]
        if cfg.sv:
            assert arr.min() >= -V and arr.max() < V
        else:
            assert arr.min() >= 0 and arr.max() < V
        arrs.append(arr)

    # ---- chunk packing
    chunks = []
    cur, tot_s = [], 0
    for b in range(NB):
        s = int(S[:, b].sum())
        if cur and tot_s + s > cfg.SLOT_CAP:
            chunks.append(cur)
            cur, tot_s = [], 0
        cur.append(b)
        tot_s += s
    if cur:
        chunks.append(cur)

    if cfg.sv:
        # The gather ucode truncates trailing-NEGATIVE indices of each call
        # (dropping those messages silently). Ensure the LAST index (stream
        # position call_end*P-1, i.e. lane 127 of the call's last slot) is
        # non-negative by swapping it with a same-(block,lane) entry.
        a0 = arrs[0]
        oV0 = np.asarray(oV[0])
        slot_blk = np.searchsorted(oV0[1:], np.arange(int(oV0[-1])),
                                   side="right")
        call_ends = set()
        for blocks in chunks:
            o0 = int(oV0[blocks[0]])
            n = int(oV0[blocks[-1] + 1]) - o0
            s = 0
            while s < n:
                m = min(MAX_CALL_SLOTS, n - s)
                call_ends.add(o0 + s + m - 1)
                s += m
        for e in sorted(call_ends):
            b = int(slot_blk[e])
            lo, hi = int(oV0[b]), int(oV0[b + 1])
            lane = a0[:, lo * P + P - 1: hi * P: P]   # [NC, S[b]] lane-127
            for c in range(NC):
                if lane[c, e - lo] < 0:
                    cand = [j for j in range(hi - lo)
                            if lane[c, j] >= 0 and (lo + j) not in call_ends]
                    assert cand, (e, b, c)
                    j = cand[0]
                    lane[c, e - lo], lane[c, j] = lane[c, j], lane[c, e - lo]

    def wrap16(arr):   # stream position q -> [q%16, q//16], replicated 8x
        w = arr.reshape(NC, -1, 16).transpose(0, 2, 1).astype(np.int16)
        return np.tile(w, (1, 8, 1))

    idxs = [wrap16(a) for a in arrs]

    # ---- per-position node data
    xp = np.zeros((TOT, cfg.K_PAD), dtype=np.float16)
    xp[pos_of, : cfg.F_IN] = np.asarray(x, np.float32).astype(np.float16)
    dinv_pos = np.zeros(TOT, dtype=np.float32)
    dinv_pos[pos_of] = dinv
    rdinv_pos = np.zeros(TOT, dtype=np.float32)
    rdinv_pos[pos_of] = rdinv

    W1p = np.zeros((cfg.K_PAD, cfg.F_HID), dtype=np.float16)
    W1p[: cfg.F_IN] = np.asarray(W1, np.float32).astype(np.float16)
    W2p = np.zeros((cfg.F_HID, P), dtype=np.float16)
    W2p[:, : cfg.F_OUT] = np.asarray(W2, np.float32).astype(np.float16)
    b1row = np.asarray(b1, np.float32).astype(np.float16).reshape(1, cfg.F_HID)
    b2row = np.zeros((1, P), dtype=np.float16)
    b2row[0, : cfg.F_OUT] = np.asarray(b2, np.float32).astype(np.float16)
    b2row[0, cfg.F_OUT: cfg.FO_PAD] = -60000.0
    ident = np.eye(P, dtype=np.float16)

    in_maps = []
    for c in range(NC):
        xc = xp[c * NPC_PAD:(c + 1) * NPC_PAD]
        m = {
            "xT": np.ascontiguousarray(xc.T).reshape(cfg.KC, P, NPC_PAD),
            "W1p": W1p.reshape(cfg.KC, P, cfg.F_HID),
            "W2p": W2p,
            "b1row": b1row,
            "b2row": b2row,
            "dinvc": np.ascontiguousarray(
                dinv_pos[c * NPC_PAD:(c + 1) * NPC_PAD].reshape(NB, P).T),
            "rdinvT": rdinv_pos[c * NPC_PAD:(c + 1) * NPC_PAD]
                      .reshape(1, NPC_PAD).astype(np.float16),
            "ident": ident,
        }
        for v in range(NV):
            m[f"idx{v}"] = idxs[v][c]
        in_maps.append(m)

    sched = {
        "S": S.tolist(), "oV": oV.tolist(), "TV": [int(t) for t in TV],
        "chunks": chunks,
    }
    return in_maps, sched, pos_of


def _dma_gather_narrow(gps, out_ap, in_ap, idxs_ap, num_idxs, num_idxs_reg,
                       elem_size, elem_step, queue_num=0):
    """dma_gather without the %256B elem-size restriction (non-transpose,
    DRAM source). The 256B-granularity constraint is on the row stride
    (stride_bytes_256 field), not the payload size. HW-verified (smoke9)."""
    from concourse import ap_utils
    gps._assert_queue_num(queue_num)
    assert idxs_ap.dtype == mybir.dt.int16
    assert in_ap.space == bass.MemorySpace.DRAM
    assert in_ap.dtype == out_ap.dtype
    assert ap_utils.ap_is_contiguous(out_ap.ap[1:])
    assert ap_utils.ap_is_contiguous(idxs_ap.ap[1:])
    assert in_ap.ap[0][0] == elem_step
    stride_bytes = elem_step * mybir.dt.size(in_ap.dtype)
    assert stride_bytes % 256 == 0 and stride_bytes // 256 < 256
    assert in_ap.ap[-1][1] == elem_size
    assert out_ap.ap[-1][1] == elem_size
    assert num_idxs % P == 0
    assert out_ap.ap[0][1] * out_ap.ap[1][1] == num_idxs
    _in_ap = gps.lower_ap_dma(in_ap, for_custom_bir_dma=True)
    _idxs_ap = gps.lower_ap(idxs_ap)
    _out_ap = gps.lower_ap(out_ap)
    return gps.add_instruction(
        mybir.InstDMAGatherAnt(
            name=gps.bass.get_next_instruction_name(),
            ins=[*_in_ap, _idxs_ap, gps.lower_val_access(gps.to_reg(num_idxs_reg))],
            outs=[_out_ap],
            transpose=False, num_idxs=num_idxs, elem_size=elem_size,
            stride_bytes_256=stride_bytes // 256, gen_mode=0,
            single_packet=True, queue_num=queue_num,
            sbuf_tokens_per_rank=0, sbuf_free_dim_per_rank=0,
            sbuf_free_dim_pad_per_rank=0, sbuf_byte_offset=0,
        ))


def _build_program(cfg, sch):
    NB, NPC_PAD, TOT = cfg.NB, cfg.NPC_PAD, cfg.TOT
    FH, KC, NV = cfg.F_HID, cfg.KC, cfg.NV
    S = sch["S"]
    oV = sch["oV"]

    nc = bacc.Bacc("TRN2", target_bir_lowering=False, debug=False,
                   num_devices=1 if cfg.single_core else NC,
                   num_swdge_queues=4)
    xT_in = nc.dram_tensor("xT", [KC, P, NPC_PAD], F16, kind="ExternalInput")
    W1_in = nc.dram_tensor("W1p", [KC, P, FH], F16, kind="ExternalInput")
    W2_in = nc.dram_tensor("W2p", [FH, P], F16, kind="ExternalInput")
    b1_in = nc.dram_tensor("b1row", [1, FH], F16, kind="ExternalInput")
    b2_in = nc.dram_tensor("b2row", [1, P], F16, kind="ExternalInput")
    dinv_in = nc.dram_tensor("dinvc", [P, NB], F32, kind="ExternalInput")
    rdinv_in = nc.dram_tensor("rdinvT", [1, NPC_PAD], F16, kind="ExternalInput")
    idx_in = [nc.dram_tensor(f"idx{v}", [P, sch["TV"][v] * P // 16], I16,
                             kind="ExternalInput") for v in range(NV)]
    id_in = nc.dram_tensor("ident", [P, P], F16, kind="ExternalInput")
    out_dram = nc.dram_tensor("out", [NPC_PAD, cfg.FO_PAD], F32,
                              kind="ExternalOutput")

    rg = [list(range(NC))]

    with tile.TileContext(nc) as tc:
        with tc.tile_pool(name="sb", bufs=1) as sb, \
             tc.tile_pool(name="ps", bufs=1, space="PSUM") as ps, \
             tc.tile_pool(name="dram", bufs=1, space="DRAM") as dram:

            # --- constant loads (once, outside the repeat loop) -----------
            W1_t = []
            for k in range(KC):
                wk = sb.tile([P, FH], F16, name=f"W1_t{k}")
                nc.sync.dma_start(wk[:], W1_in.ap()[k])
                W1_t.append(wk)
            W2_t = sb.tile([FH, P], F16, name="W2_t")
            nc.sync.dma_start(W2_t[:], W2_in.ap())
            b1_t = sb.tile([1, FH], F16, name="b1_t")
            nc.sync.dma_start(b1_t[:], b1_in.ap())
            b2_t = sb.tile([1, P], F16, name="b2_t")
            nc.sync.dma_start(b2_t[:], b2_in.ap())
            dinv_t = sb.tile([P, NB], F32, name="dinv_t")
            nc.sync.dma_start(dinv_t[:], dinv_in.ap())
            rdinv_t = sb.tile([1, NPC_PAD], F16, name="rdinv_t")
            nc.sync.dma_start(rdinv_t[:], rdinv_in.ap())
            idx_t = []
            for v in range(NV):
                it = sb.tile([P, sch["TV"][v] * P // 16], I16, name=f"idx_t{v}")
                nc.sync.dma_start(it[:], idx_in[v].ap())
                idx_t.append(it)
            id_t = sb.tile([P, P], F16, name="id_t")
            nc.sync.dma_start(id_t[:], id_in.ap())

            z1_all = sb.tile([P, NB, FH], F16, name="z1_all")
            if cfg.phase >= 2:
                h1_t = sb.tile([P, NB, FH], F16, name="h1_t")
            if cfg.phase >= 3:
                z2_all = sb.tile([P, NB, P], F16, name="z2_all")
            if cfg.phase >= 4:
                out_all = sb.tile([P, NB, cfg.FO_PAD], F32, name="out_all")

            # --- generic gather + segment-reduce phase -------------------
            qctr = [0]

            def agg_phase(table, fw_row, fw_fetch, brow_t, fo, consume,
                          selfrow=None):
                """table rows are [*, fw_row] f16 (256B-multiple stride);
                each descriptor fetches the first fw_fetch cols; reduce fo
                cols per block into PSUM; consume(b, psum) finishes it.
                selfrow: per-block SBUF rows added once into each block's
                PSUM group (self-loop contribution when cfg.no_self)."""
                if cfg.sv:
                    views = [table[cfg.BASES[0]:TOT, 0:fw_fetch]]
                else:
                    views = [table[cfg.BASES[v]:cfg.BASES[v] + cfg.VIEW_ROWS,
                                   0:fw_fetch] if NV > 1
                             else table[:, 0:fw_fetch]
                             for v in range(NV)]

                def gather_stream(g_tile, g_off, view, it, o0, n_slots):
                    sslot = 0
                    while sslot < n_slots:
                        m = min(MAX_CALL_SLOTS, n_slots - sslot)
                        _dma_gather_narrow(
                            nc.gpsimd,
                            out_ap=g_tile[:, g_off + sslot:g_off + sslot + m, :],
                            in_ap=view,
                            idxs_ap=it[:, (o0 + sslot) * 8:(o0 + sslot + m) * 8],
                            num_idxs=m * P, num_idxs_reg=m * P,
                            elem_size=fw_fetch, elem_step=fw_row,
                            queue_num=qctr[0] % 4)
                        qctr[0] += 1
                        sslot += m

                for ci, blocks in enumerate(sch["chunks"]):
                    b0, b1_ = blocks[0], blocks[-1]
                    nS = [oV[v][b1_ + 1] - oV[v][b0] for v in range(NV)]
                    g = sb.tile([P, sum(nS), fw_fetch], F16, tag="g", bufs=2,
                                name=f"g{fw_fetch}_{ci}")
                    go = np.r_[0, np.cumsum(nS)]
                    for v in range(NV):
                        if nS[v]:
                            gather_stream(g, int(go[v]), views[v], idx_t[v],
                                          oV[v][b0], nS[v])
                    for b in blocks:
                        pag = ps.tile([P, fo], F32, tag="ps", bufs=4,
                                      name=f"pag{fw_fetch}_{b}")
                        first = True
                        if selfrow is not None:
                            nc.tensor.matmul(
                                out=pag[:], lhsT=id_t[:],
                                rhs=selfrow[:, b, 0:fo],
                                start=first, stop=False)
                            first = False
                        for v in range(NV):
                            for j in range(S[v][b]):
                                nc.tensor.matmul(
                                    out=pag[:], lhsT=id_t[:],
                                    rhs=g[:, int(go[v]) + oV[v][b] - oV[v][b0]
                                          + j, 0:fo],
                                    start=first, stop=False)
                                first = False
                        nc.tensor.matmul(
                            out=pag[:], lhsT=rdinv_t[:, b * P:(b + 1) * P],
                            rhs=brow_t[:, 0:fo], start=first, stop=True)
                        consume(b, pag)

            def l1_consume(b, pag):
                nc.scalar.activation(h1_t[:, b, :], pag[:],
                                     mybir.ActivationFunctionType.Relu,
                                     bias=0.0, scale=dinv_t[:, b:b + 1])

            def l2_consume(b, pag):
                fo = cfg.FO_PAD
                m0 = sb.tile([P, 1], F32, tag="m0", bufs=3, name=f"m0_{b}")
                nc.vector.tensor_reduce(m0[:], pag[:], mybir.AxisListType.X,
                                        mybir.AluOpType.max)
                mneg = sb.tile([P, 1], F32, tag="mneg", bufs=3, name=f"mn{b}")
                nc.vector.tensor_scalar(mneg[:], m0[:], dinv_t[:, b:b + 1],
                                        -1.0, mybir.AluOpType.mult,
                                        mybir.AluOpType.mult)
                e_t = sb.tile([P, fo], F32, tag="e", bufs=3, name=f"e{b}")
                s_t = sb.tile([P, 1], F32, tag="s", bufs=3, name=f"s{b}")
                nc.scalar.activation(e_t[:], pag[:],
                                     mybir.ActivationFunctionType.Exp,
                                     bias=mneg[:], scale=dinv_t[:, b:b + 1],
                                     accum_out=s_t[:])
                lse = sb.tile([P, 1], F32, tag="lse", bufs=3, name=f"ls{b}")
                nc.scalar.activation(lse[:], s_t[:],
                                     mybir.ActivationFunctionType.Ln)
                c_t = sb.tile([P, 1], F32, tag="c", bufs=3, name=f"c{b}")
                nc.vector.tensor_tensor(out=c_t[:], in0=lse[:], in1=mneg[:],
                                        op=mybir.AluOpType.subtract)
                nc.vector.tensor_scalar(out_all[:, b, :], pag[:],
                                        dinv_t[:, b:b + 1],
                                        c_t[:], mybir.AluOpType.mult,
                                        mybir.AluOpType.subtract)

            def all_gather(bounce, table):
                if cfg.single_core or cfg.no_cc:
                    nc.sync.dma_start(table[0:NPC_PAD, :], bounce[:])
                else:
                    nc.gpsimd.collective_compute(
                        "AllGather", mybir.AluOpType.bypass, replica_groups=rg,
                        ins=[bounce.opt()], outs=[table.opt()])

            for rep in range(cfg.repeat):
                aspace = "Shared" if cfg.shared_tables else "Local"
                bounce1 = dram.tile([NPC_PAD, FH], F16, name=f"bounce1_{rep}")
                table1 = dram.tile([TOT, FH], F16, name=f"table1_{rep}",
                                   addr_space=aspace)
                bounce2 = dram.tile([NPC_PAD, P], F16, name=f"bounce2_{rep}")
                table2 = dram.tile([TOT, P], F16, name=f"table2_{rep}",
                                   addr_space=aspace)
                # --- layer-1 dense transform: z1' = dinv * (x @ W1) ------
                xT_t, xT_free = [], []
                for k in range(KC):
                    xk, xfree = tc.tile([P, NPC_PAD], F16, name=f"xT_t{k}")
                    nc.sync.dma_start(xk[:], xT_in.ap()[k])
                    xT_t.append(xk)
                    xT_free.append(xfree)
                for b in range(NB):
                    psz = ps.tile([P, FH], F32, tag="ps", bufs=4,
                                  name=f"psz{b}")
                    for k in range(KC):
                        nc.tensor.matmul(out=psz[:],
                                         lhsT=xT_t[k][:, b * P:(b + 1) * P],
                                         rhs=W1_t[k][:],
                                         start=(k == 0), stop=(k == KC - 1))
                    nc.scalar.activation(z1_all[:, b, :], psz[:],
                                         mybir.ActivationFunctionType.Copy,
                                         bias=0.0, scale=dinv_t[:, b:b + 1])
                nc.sync.dma_start(
                    bounce1[:].rearrange("(nb p) f -> p nb f", p=P), z1_all[:])
                for f in reversed(xT_free):
                    f()

                if cfg.phase >= 1:
                    all_gather(bounce1, table1)

                if cfg.phase <= 1:
                    dbg = sb.tile([P, cfg.FO_PAD], F32, name="dbgout")
                    nc.gpsimd.memset(dbg[:], 0.0)
                    for b in range(NB):
                        nc.sync.dma_start(out_dram.ap()[b * P:(b + 1) * P, :],
                                          dbg[:])

                # --- layer-1 aggregation -> h1 ---------------------------
                if cfg.phase >= 2:
                    import os as _os
                    _fw = int(_os.environ.get("GCN_L1FW", str(FH)))
                    _sr = z1_all if cfg.no_self else None
                    if _fw != FH:
                        def _l1c(b, pag):
                            nc.scalar.activation(
                                h1_t[:, b, 0:_fw], pag[:],
                                mybir.ActivationFunctionType.Relu,
                                bias=0.0, scale=dinv_t[:, b:b + 1])
                        agg_phase(table1, FH, _fw, b1_t, _fw, _l1c,
                                  selfrow=_sr)
                    else:
                        agg_phase(table1, FH, FH, b1_t, FH, l1_consume,
                                  selfrow=_sr)

                if cfg.phase == 2:
                    for b in range(NB):
                        o_t = sb.tile([P, cfg.FO_PAD], F32, tag="o", bufs=3,
                                      name=f"dbg{b}")
                        nc.vector.tensor_copy(o_t[:], h1_t[:, b, 0:cfg.FO_PAD])
                        nc.sync.dma_start(out_dram.ap()[b * P:(b + 1) * P, :],
                                          o_t[:])

                # --- layer-2 dense transform: z2' = dinv * (h1 @ W2) -----
                for b in range(NB if cfg.phase >= 3 else 0):
                    pst = ps.tile([P, P], F16, tag="pst", bufs=2,
                                  name=f"pst{b}")
                    nc.tensor.transpose(out=pst[:], in_=h1_t[:, b, :],
                                        identity=id_t[:])
                    h1T = sb.tile([P, P], F16, tag="h1T", bufs=3,
                                  name=f"h1T{b}")
                    nc.scalar.activation(h1T[:], pst[:],
                                         mybir.ActivationFunctionType.Copy)
                    psz2 = ps.tile([P, P], F32, tag="ps", bufs=4,
                                   name=f"psz2{b}")
                    nc.tensor.matmul(out=psz2[:], lhsT=h1T[:], rhs=W2_t[:],
                                     start=True, stop=True)
                    nc.scalar.activation(z2_all[:, b, :], psz2[:],
                                         mybir.ActivationFunctionType.Copy,
                                         bias=0.0, scale=dinv_t[:, b:b + 1])

                if cfg.phase >= 3:
                    nc.sync.dma_start(
                        bounce2[:].rearrange("(nb p) f -> p nb f", p=P),
                        z2_all[:])
                    all_gather(bounce2, table2)

                if cfg.phase == 3:
                    dbg3 = sb.tile([P, cfg.FO_PAD], F32, name="dbgout3")
                    nc.gpsimd.memset(dbg3[:], 0.0)
                    for b in range(NB):
                        nc.sync.dma_start(out_dram.ap()[b * P:(b + 1) * P, :],
                                          dbg3[:])

                # --- layer-2 aggregation + log_softmax -------------------
                if cfg.phase >= 4:
                    agg_phase(table2, P, cfg.FO_PAD, b2_t, cfg.FO_PAD,
                              l2_consume,
                              selfrow=z2_all if cfg.no_self else None)
                    nc.sync.dma_start(
                        out_dram.ap().rearrange("(nb p) f -> p nb f", p=P),
                        out_all[:])

    nc.compile()
    return nc


LAST_RESULTS = None


def kernel(x, W1, b1, W2, b2, edge_index):
    global LAST_RESULTS
    import os
    import time
    cfg = Cfg(N=50000, F_IN=500, F_HID=128, F_OUT=47,
              phase=int(os.environ.get("GCN_PHASE", "4")),
              repeat=int(os.environ.get("GCN_REPEAT", "1")),
              no_cc=os.environ.get("GCN_NO_CC", "0") == "1",
              shared_tables=os.environ.get("GCN_SHARED", "1") == "1",
              sv=os.environ.get("GCN_SV", "1") == "1",
              no_self=os.environ.get("GCN_NOSELF", "1") == "1")
    in_maps, sched, pos_of = _preprocess(x, W1, b1, W2, b2, edge_index, cfg)
    nc = _build_program(cfg, sched)
    res = None
    for attempt in range(3):
        try:
            res = bass_utils.run_bass_kernel_spmd(
                nc, in_maps, core_ids=list(range(NC)))
            break
        except Exception:
            if attempt == 2:
                raise
            time.sleep(5)
    LAST_RESULTS = res
    alls = np.concatenate([np.asarray(res.results[c]["out"])
                           for c in range(NC)], axis=0)
    return alls[pos_of, : cfg.F_OUT].astype(np.float32)

